# revision 1
# baseline (speedup 1.0000x reference)
"""CSABlock Trainium2 kernel, plan B: feature reading + maxpool + phi/g
production split between the two cores of each sample (h-halves), exchanged
via 2-core AllGather. Attention stays query-split.

Core = 2n + h. Each core loads only its h-half of feature[n] (18.9MB),
maxpools it, computes phi/gT for its half, AllGathers the halves blockwise,
then runs the full 2048x4096 attention for its query half.
"""

import numpy as np
import ml_dtypes

import concourse.bass as bass
import concourse.mybir as mybir
import concourse.tile as tile
from concourse import bacc

F32 = mybir.dt.float32
F32R = mybir.dt.float32r
BF16 = mybir.dt.bfloat16

C = 256
IC = 128
D = 9
HW = 4096
Q = 2048          # query pixels per core, also local key pixels per core
NLB = 2           # local pixel blocks (of 1024) for compute pipelining
BLK = Q // NLB    # 1024
NMP = 4           # maxpool stream blocks over the local half
MPB = Q // NMP    # 512 px
KCH = 32          # key chunks of 128 px over the FULL image
CPB = BLK // 128  # chunks per 1024-block = 8
EXP_BIAS = -30.0
EPS = 1e-5
GROUPS = [[0, 1], [2, 3], [4, 5], [6, 7]]

AF = mybir.ActivationFunctionType


def build(nc):
    feat_d = nc.dram_tensor("feat", [2, 128, D, NMP, MPB], F32, kind="ExternalInput")
    center_d = nc.dram_tensor("center", [2, 128, Q], F32R, kind="ExternalInput")
    wthT_d = nc.dram_tensor("wthT", [2, 128, 128], F32R, kind="ExternalInput")
    wphT_d = nc.dram_tensor("wphT", [2, 128, 128], F32R, kind="ExternalInput")
    wgT_d = nc.dram_tensor("wgT", [2, 128, 128], F32R, kind="ExternalInput")
    wwT_d = nc.dram_tensor("wwT", [2, 128, 128], F32R, kind="ExternalInput")
    bn_d = nc.dram_tensor("bnpack", [128, 8], F32, kind="ExternalInput")
    mask_d = nc.dram_tensor("mask", [128, 2], F32, kind="ExternalInput")
    out_d = nc.dram_tensor("out", [2, 128, Q], F32, kind="ExternalOutput")

    # packed exchange buffers (per block): [:, 0:BLK] = phi (f32r),
    # [:, BLK:PK] = gT as bf16 pairs. Each core contributes its packed block
    # to the PARTNER's ReduceScatter slot (mask-selected, zero in own slot),
    # so rs_d always holds exactly the partner's half -> SPMD-uniform.
    PK = BLK + BLK // 2
    pb2_d = [nc.dram_tensor(f"pb2_{lb}", [2, 128, PK], F32R) for lb in range(NLB)]
    rs_d = [nc.dram_tensor(f"rs{lb}", [128, PK], F32R) for lb in range(NLB)]

    with tile.TileContext(nc) as tc:
        with (
            tc.tile_pool(name="persist", bufs=1) as pp,
            tc.tile_pool(name="tmp", bufs=4) as tp,
            tc.tile_pool(name="fstream", bufs=2) as fp,
            tc.tile_pool(name="et", bufs=4) as ep,
            tc.tile_pool(name="zt", bufs=2) as zp,
            tc.tile_pool(name="psacc", bufs=1, space="PSUM") as pacc,
            tc.tile_pool(name="pssc", bufs=2, space="PSUM") as psc,
            tc.tile_pool(name="dram", bufs=1, space="DRAM") as dp,
        ):
            # ---- small loads ----
            center_sb = pp.tile([128, 2, Q], F32R)
            wthT = pp.tile([128, 2, 128], F32R)
            wphT = pp.tile([128, 2, 128], F32R)
            wgT = pp.tile([128, 2, 128], F32R)
            wwT = pp.tile([128, 2, 128], F32R)
            bn = pp.tile([128, 8], F32)
            for cc in range(2):
                nc.sync.dma_start(out=center_sb[:, cc, :], in_=center_d[cc])
                nc.sync.dma_start(out=wthT[:, cc, :], in_=wthT_d[cc])
                nc.sync.dma_start(out=wphT[:, cc, :], in_=wphT_d[cc])
                nc.sync.dma_start(out=wgT[:, cc, :], in_=wgT_d[cc])
                nc.sync.dma_start(out=wwT[:, cc, :], in_=wwT_d[cc])
            nc.sync.dma_start(out=bn[:], in_=bn_d[:])
            maskv = pp.tile([128, 2], F32)
            nc.sync.dma_start(out=maskv[:], in_=mask_d[:])

            ones32 = pp.tile([128, 1], F32)
            nc.vector.memset(ones32, 1.0)
            ones = pp.tile([128, 1], F32R)
            nc.vector.tensor_copy(ones, ones32)
            eps_t = pp.tile([128, 1], F32)
            nc.vector.memset(eps_t, EPS)
            expb = pp.tile([128, 1], F32)
            nc.vector.memset(expb, EXP_BIAS)

            # ---- BN folding ----
            sc_th = pp.tile([128, 1], F32)
            bi_th = pp.tile([128, 1], F32)
            sc_ph = pp.tile([128, 1], F32)
            bi_ph = pp.tile([128, 1], F32)
            for (o, sc_t, bi_t) in ((0, sc_th, bi_th), (4, sc_ph, bi_ph)):
                lnv = tp.tile([128, 1], F32, tag="bntmp")
                nc.scalar.activation(lnv, bn[:, o + 3 : o + 4], AF.Ln, bias=eps_t[:])
                rsq = tp.tile([128, 1], F32, tag="bntmp")
                nc.scalar.activation(rsq, lnv, AF.Exp, scale=-0.5)
                nc.vector.tensor_mul(sc_t, bn[:, o : o + 1], rsq)
                ms = tp.tile([128, 1], F32, tag="bntmp")
                nc.vector.tensor_mul(ms, bn[:, o + 2 : o + 3], sc_t)
                nc.vector.tensor_sub(bi_t, bn[:, o + 1 : o + 2], ms)

            # ---- theta ----
            theta = pp.tile([128, Q], F32R)
            ps_th = pacc.tile([128, Q], F32, tag="acc")
            for cc in range(2):
                for qc in range(Q // 512):
                    nc.tensor.matmul(
                        ps_th[:, qc * 512 : (qc + 1) * 512],
                        lhsT=wthT[:, cc, :],
                        rhs=center_sb[:, cc, qc * 512 : (qc + 1) * 512],
                        start=(cc == 0),
                        stop=(cc == 1),
                    )
            nc.scalar.activation(theta, ps_th, AF.Relu, bias=bi_th[:], scale=sc_th[:])

            # ---- persistent ----
            x_sb = pp.tile([128, 2, Q], F32R)     # local-half maxpool
            pk_phi = pp.tile([128, NLB, BLK], F32R)   # local phi blocks
            pk_g = pp.tile([128, NLB, BLK], BF16)     # local gT blocks
            phi_rem = pp.tile([128, NLB, BLK], F32R)
            gT_rem = pp.tile([128, NLB, BLK], BF16)
            pkm_phi = pp.tile([128, 2, BLK], F32R)    # mask-scaled copies
            pkm_g = pp.tile([128, 2, BLK], BF16)
            zacc = pp.tile([128, Q], F32R)

            out_acc = pacc.tile([128, Q], F32, tag="acc")

            nchunks = 2 * NLB * CPB
            state = {"idx": 0, "etq": []}

            def attention_chunks(phis, gts):
                for j in range(CPB):
                    et = ep.tile([128, Q], BF16, tag="et")
                    for qhalf in range(2):
                        s_ps = psc.tile([128, 1024], F32, tag="sc")
                        for qc in range(2):
                            o = qhalf * 1024 + qc * 512
                            nc.tensor.matmul(
                                s_ps[:, qc * 512 : (qc + 1) * 512],
                                lhsT=phis[:, j * 128 : (j + 1) * 128],
                                rhs=theta[:, o : o + 512],
                                start=True,
                                stop=True,
                            )
                        nc.scalar.activation(
                            et[:, qhalf * 1024 : (qhalf + 1) * 1024],
                            s_ps, AF.Exp, bias=expb[:],
                        )
                    idx = state["idx"]
                    for qc in range(4):
                        nc.tensor.matmul(
                            out_acc[:, qc * 512 : (qc + 1) * 512],
                            lhsT=gts[:, j * 128 : (j + 1) * 128],
                            rhs=et[:, qc * 512 : (qc + 1) * 512],
                            start=(idx == 0),
                            stop=(idx == nchunks - 1),
                        )
                    state["etq"].append(et)
                    if len(state["etq"]) == 4:
                        e0, e1, e2, e3 = state["etq"]
                        state["etq"] = []
                        p0 = zp.tile([128, Q], BF16, tag="zpair")
                        p1 = zp.tile([128, Q], BF16, tag="zpair")
                        nc.vector.tensor_add(p0, e0, e1)
                        nc.vector.tensor_add(p1, e2, e3)
                        q0 = zp.tile([128, Q], BF16, tag="zquad")
                        nc.vector.tensor_add(q0, p0, p1)
                        if idx == 3:
                            nc.vector.tensor_copy(zacc, q0)
                        else:
                            nc.vector.tensor_add(zacc, zacc, q0)
                    state["idx"] = idx + 1

            # ---- local half: maxpool + phi + gT + exchange + LOCAL attention,
            # interleaved per block so the in-order PE stream can start the
            # attention as soon as block 0 is produced.
            for lb in range(NLB):
                bs = slice(lb * BLK, (lb + 1) * BLK)
                for cc in range(2):
                    for m in range(BLK // MPB):
                        mb = lb * (BLK // MPB) + m
                        ms = slice(mb * MPB, (mb + 1) * MPB)
                        ft = fp.tile([128, D, MPB], F32, tag="feat")
                        nc.sync.dma_start(out=ft[:], in_=feat_d[cc, :, :, mb, :])
                        tmp = tp.tile([128, MPB], F32, tag="mp")
                        nc.vector.tensor_max(tmp, ft[:, 0, :], ft[:, 1, :])
                        for d in range(2, D - 1):
                            nc.vector.tensor_max(tmp, tmp, ft[:, d, :])
                        nc.vector.tensor_max(x_sb[:, cc, ms], tmp, ft[:, D - 1, :])

                ps_ph = psc.tile([128, BLK], F32, tag="sc")
                for cc in range(2):
                    for sub in range(BLK // 512):
                        nc.tensor.matmul(
                            ps_ph[:, sub * 512 : (sub + 1) * 512],
                            lhsT=wphT[:, cc, :],
                            rhs=x_sb[:, cc, lb * BLK + sub * 512 : lb * BLK + (sub + 1) * 512],
                            start=(cc == 0),
                            stop=(cc == 1),
                        )
                nc.scalar.activation(
                    pk_phi[:, lb, :], ps_ph, AF.Relu, bias=bi_ph[:], scale=sc_ph[:]
                )

                ps_g = psc.tile([128, BLK], F32, tag="sc")
                for j in range(CPB):
                    ks = slice(lb * BLK + j * 128, lb * BLK + (j + 1) * 128)
                    for cc in range(2):
                        nc.tensor.matmul(
                            ps_g[:, j * 128 : (j + 1) * 128],
                            lhsT=x_sb[:, cc, ks],
                            rhs=wgT[:, cc, :],
                            start=(cc == 0),
                            stop=(cc == 1),
                        )
                nc.scalar.copy(pk_g[:, lb, :], ps_g)

                # exchange this block with the partner (mask-scaled RS(add)):
                # non-feature DMAs ride the gpsimd (SWDGE) queue.
                for sslot in range(2):
                    mk = maskv[:, sslot : sslot + 1]
                    nc.scalar.mul(pkm_phi[:, sslot, :], pk_phi[:, lb, :], mk)
                    nc.scalar.mul(pkm_g[:, sslot, :], pk_g[:, lb, :], mk)
                    nc.gpsimd.dma_start(
                        out=pb2_d[lb][sslot][:, 0:BLK], in_=pkm_phi[:, sslot, :]
                    )
                    nc.gpsimd.dma_start(
                        out=pb2_d[lb][sslot][:, BLK:PK].bitcast(BF16),
                        in_=pkm_g[:, sslot, :],
                    )
                nc.gpsimd.collective_compute(
                    "ReduceScatter", mybir.AluOpType.add, replica_groups=GROUPS,
                    ins=[pb2_d[lb].ap().opt()], outs=[rs_d[lb].ap().opt()],
                )
                nc.gpsimd.dma_start(out=phi_rem[:, lb, :], in_=rs_d[lb][:, 0:BLK])
                nc.gpsimd.dma_start(
                    out=gT_rem[:, lb, :], in_=rs_d[lb][:, BLK:PK].bitcast(BF16)
                )

                # local attention for this block while the exchange is in flight
                attention_chunks(pk_phi[:, lb, :], pk_g[:, lb, :])

            # remote halves once the ReduceScatters land
            for lb in range(NLB):
                attention_chunks(phi_rem[:, lb, :], gT_rem[:, lb, :])

            # ---- softmax normalization ----
            zrow_sb = pp.tile([1, Q], F32)
            for half in range(2):
                hs = slice(half * 1024, (half + 1) * 1024)
                zrow = psc.tile([1, 1024], F32, tag="sc")
                for qc in range(2):
                    o = half * 1024 + qc * 512
                    nc.tensor.matmul(
                        zrow[:, qc * 512 : (qc + 1) * 512],
                        lhsT=ones[:, 0:1],
                        rhs=zacc[:, o : o + 512],
                        start=True,
                        stop=True,
                    )
                nc.scalar.copy(zrow_sb[:, hs], zrow)
            zb = dp.tile([1, Q], F32)
            nc.sync.dma_start(out=zb[:], in_=zrow_sb[:])
            zcols = pp.tile([128, Q // 128], F32)
            nc.sync.dma_start(out=zcols[:], in_=zb.rearrange("o (p c) -> (o p) c", p=128))
            izcols = pp.tile([128, Q // 128], F32)
            nc.vector.reciprocal(izcols, zcols)
            zbi = dp.tile([1, Q], F32)
            nc.sync.dma_start(out=zbi.rearrange("o (p c) -> (o p) c", p=128), in_=izcols[:])
            invz = pp.tile([128, Q], F32)
            zbi_b = bass.AP(
                tensor=zbi.tensor, offset=zbi.offset,
                ap=[[0, 128]] + [list(p) for p in zbi.ap[1:]],
            )
            nc.sync.dma_start(out=invz[:], in_=zbi_b)

            wsb = pp.tile([128, Q], F32R)
            nc.vector.tensor_mul(wsb, out_acc, invz)
            out_sb = pp.tile([128, 2, Q], F32)
            for oc in range(2):
                ps_o = pacc.tile([128, Q], F32, tag="acc")
                for qc in range(Q // 512):
                    nc.tensor.matmul(
                        ps_o[:, qc * 512 : (qc + 1) * 512],
                        lhsT=wwT[:, oc, :],
                        rhs=wsb[:, qc * 512 : (qc + 1) * 512],
                        start=True,
                        stop=True,
                    )
                nc.vector.tensor_add(out_sb[:, oc, :], ps_o, center_sb[:, oc, :])
                nc.sync.dma_start(out=out_d[oc], in_=out_sb[:, oc, :])


def shard_inputs(inputs):
    f32 = np.float32
    feature = np.asarray(inputs["feature"], dtype=f32)
    w_theta = np.asarray(inputs["w_theta"], dtype=f32)
    w_phi = np.asarray(inputs["w_phi"], dtype=f32)
    w_g = np.asarray(inputs["w_g"], dtype=f32)
    w_w = np.asarray(inputs["w_w"], dtype=f32)
    wthT = np.ascontiguousarray(w_theta.T.reshape(2, 128, 128))
    wphT = np.ascontiguousarray(w_phi.T.reshape(2, 128, 128))
    wgT = np.ascontiguousarray(w_g.T.reshape(2, 128, 128))
    wwT = np.ascontiguousarray(w_w.T.reshape(128, 2, 128).transpose(1, 0, 2))
    bnpack = np.ascontiguousarray(np.stack(
        [
            np.asarray(inputs["bn_theta_gamma"], f32),
            np.asarray(inputs["bn_theta_beta"], f32),
            np.asarray(inputs["bn_theta_mean"], f32),
            np.asarray(inputs["bn_theta_var"], f32),
            np.asarray(inputs["bn_phi_gamma"], f32),
            np.asarray(inputs["bn_phi_beta"], f32),
            np.asarray(inputs["bn_phi_mean"], f32),
            np.asarray(inputs["bn_phi_var"], f32),
        ],
        axis=1,
    ))

    in_maps = []
    for core in range(8):
        n, h = core // 2, core % 2
        fh = feature[n].reshape(2, 128, D, HW)[:, :, :, h * Q : (h + 1) * Q]
        feat = np.ascontiguousarray(fh.reshape(2, 128, D, NMP, MPB))
        center = np.ascontiguousarray(
            feature[n][:, D // 2 + 1].reshape(256, HW)[:, h * Q : (h + 1) * Q]
            .reshape(2, 128, Q)
        )
        mask = np.zeros((128, 2), dtype=np.float32)
        mask[:, 1 - h] = 1.0
        in_maps.append(
            dict(feat=feat, center=center, wthT=wthT, wphT=wphT, wgT=wgT,
                 wwT=wwT, bnpack=bnpack, mask=mask)
        )
    return in_maps


def unshard_output(results, N=4):
    out = np.empty((N, 256, 64, 64), dtype=np.float32)
    flat = out.reshape(N, 256, HW)
    for core in range(8):
        n, qh = core // 2, core % 2
        flat[n][:, qh * Q : (qh + 1) * Q] = results[core]["out"].reshape(256, Q)
    return out


def make_nc():
    nc = bacc.Bacc("TRN2", target_bir_lowering=False, debug=False, num_devices=8)
    build(nc)
    nc.compile()
    return nc


# ---------------------------------------------------------------------------
# Public entrypoint: full (unsharded) inputs -> full output, running the Bass
# kernel SPMD across the 8 NeuronCores.
# ---------------------------------------------------------------------------
from concourse.bass_utils import run_bass_kernel_spmd

_NC_CACHE = []


def _get_nc():
    if not _NC_CACHE:
        _NC_CACHE.append(make_nc())
    return _NC_CACHE[0]


def kernel(**inputs):
    nc = _get_nc()
    in_maps = shard_inputs(inputs)
    res = run_bass_kernel_spmd(nc, in_maps, list(range(8)))
    return unshard_output(res.results)

